# revision 28
# baseline (speedup 1.0000x reference)
"""Trainium2 Bass kernel for CapsuleLayer dynamic routing (B=128, I=1152, J=128, K=32, D=32).

Strategy
--------
Data-parallel over batch: 16 samples per core x 8 cores. u_hat is never
materialized; the routing math is factorized exactly as in the baseline:

    y[s,k,j]  = sum_i c[s,i,k] x[s,i,j]
    s[s,k,d]  = sum_j y[s,k,j] W[j,k,d]
    v         = squash(s)
    t[s,j,k]  = sum_d W[j,k,d] v[s,k,d]
    b[s,i,k]  = sum_j x[s,i,j] (t0+..+t_it)[s,j,k]   (linear in t -> t accumulates
                                                      in PSUM, no b state kept)

PE orientation is flipped vs the baseline: the small per-sample matrices
(c-chunks / t, 32 columns) are the matmul *stationary* (cheap LDWEIGHTS) and x
is the *moving* operand (N=128..384), with 4x column tiling running 4 samples
concurrently in 32-column strips of the PE array. Outputs land transposed
(b^T, y^T); b^T is fixed by fusing softmax's exp (ACT, PSUM->SBUF) with PE
128x128 transposes, y^T by one PE transpose per sample group. Softmax
normalization and squash run on DVE; sqrt is computed as exp(0.5*ln(x)) so ACT
only ever uses one table set (no ACT_TABLE_LOAD thrash). s-matmuls are
column-tiled into a [128, K/4, D] strip layout (sample s of strip t lives at
partition 32t+s, capsule k = 4*kq + t).
"""
import numpy as np
import ml_dtypes
from contextlib import ExitStack

import concourse.bass as bass
import concourse.bacc as bacc_mod
import concourse.mybir as mybir
import concourse.tile as tile
from concourse.bass_utils import run_bass_kernel_spmd
from concourse.masks import make_identity

B, I, J, K, D = 128, 1152, 128, 32, 32
NCORES = 8
S = B // NCORES          # 16 samples per core
CH = I // 128            # 9 chunks of the input-capsule axis
G = 4                    # sample groups per core
SG = S // G              # 4 samples per group (one per 32-col PE strip)
KQ = K // 4              # 8 capsules per strip in the s/v layout
NSEC = 3                 # b^T i-axis sections (384 columns each)
SEC = I // NSEC
NUM_ROUTING = 3
EPS = 1e-7
F32 = mybir.dt.float32
F32R = mybir.dt.float32r
BF16 = mybir.dt.bfloat16
AX = mybir.AxisListType.X
ADD = mybir.AluOpType.add
MULT = mybir.AluOpType.mult
EXP = mybir.ActivationFunctionType.Exp
U32 = mybir.dt.uint32
A_LSR = mybir.AluOpType.logical_shift_right
A_SUB = mybir.AluOpType.subtract

_PROGRAM = None
DEBUG = False
STAGE = 6  # debug truncation: 1=it0, 2=+t0, 3=+p1 b/exp, 4=+p1 smy, 5=+p1 s/t, 6=full


def _squash(nc, sqp, s_ps):
    """vsb = squash(s_ps) along d. s_ps: [128, KQ, D] PSUM (strip layout).

    Pure-DVE: rsqrt via the 0x5F3759DF bit trick (fp-domain magic subtract,
    exact enough for a seed) + 2 Newton iterations. No ACT involvement, so
    the scalar engine only ever loads the softmax Exp table set once.
    scale = ss * rsqrt(ss) / (1 + ss); the reference's eps only matters at
    ss ~ eps where |v| < 1e-4 absolute, far under tolerance.
    """
    sq = sqp.tile([128, KQ, D], F32, tag="sq")
    nc.scalar.activation(out=sq, in_=s_ps,
                         func=mybir.ActivationFunctionType.Square)
    ss = sqp.tile([128, KQ], F32, tag="ss")
    nc.vector.tensor_reduce(out=ss, in_=sq, axis=AX, op=ADD)
    # floor ss so the Newton iterations can't overflow (ss=0 -> seed^2 = inf)
    nc.vector.tensor_scalar(out=ss, in0=ss, scalar1=1e-12, scalar2=None,
                            op0=mybir.AluOpType.max)
    ib = sqp.tile([128, KQ], U32, tag="ib")
    nc.vector.tensor_scalar(out=ib, in0=ss.bitcast(U32), scalar1=1,
                            scalar2=None, op0=A_LSR)
    nc.vector.tensor_scalar(out=ib, in0=ib, scalar1=float(0x5F3759DF),
                            scalar2=-1.0, op0=A_SUB, op1=MULT)
    y = sqp.tile([128, KQ], F32, tag="yns")
    t1 = sqp.tile([128, KQ], F32, tag="t1ns")
    rs = ib.bitcast(F32)
    for _ in range(2):
        nc.vector.tensor_mul(t1, rs, rs)
        nc.vector.tensor_mul(t1, t1, ss)
        nc.vector.tensor_scalar(out=t1, in0=t1, scalar1=-0.5, scalar2=1.5,
                                op0=MULT, op1=ADD)
        nc.vector.tensor_mul(y, rs, t1)
        rs = y
    den = sqp.tile([128, KQ], F32, tag="den")
    nc.vector.tensor_scalar(out=den, in0=ss, scalar1=1.0, scalar2=None,
                            op0=ADD)
    rden = sqp.tile([128, KQ], F32, tag="rden")
    nc.vector.reciprocal(out=rden, in_=den)
    sc = sqp.tile([128, KQ], F32, tag="sc")
    nc.vector.tensor_mul(sc, ss, y)
    nc.vector.tensor_mul(sc, sc, rden)
    vsb = sqp.tile([128, KQ, D], F32, tag="vsb")
    nc.vector.tensor_tensor(out=vsb, in0=s_ps,
                            in1=sc.unsqueeze(-1).broadcast_to([128, KQ, D]),
                            op=MULT)
    return vsb


def _build_program():
    nc = bacc_mod.Bacc("TRN2", target_bir_lowering=False, debug=False,
                       num_devices=NCORES)
    xa_d = nc.dram_tensor("xa", [128, S, CH, 128], BF16, kind="ExternalInput")
    xb_d = nc.dram_tensor("xb", [128, S, CH * 128], BF16, kind="ExternalInput")
    wr_d = nc.dram_tensor("wr", [128, K, D], F32, kind="ExternalInput")
    wt_d = nc.dram_tensor("wt", [32, K, 128], BF16, kind="ExternalInput")
    y0_d = nc.dram_tensor("y0", [128, S], F32, kind="ExternalInput")
    idz_d = nc.dram_tensor("idz", [128, 132], BF16, kind="ExternalInput")
    idf_d = nc.dram_tensor("idf", [128, 128], F32, kind="ExternalInput")
    id16_d = nc.dram_tensor("id16", [16, 16], F32, kind="ExternalInput")
    v_d = nc.dram_tensor("vout", [S, K, D], F32, kind="ExternalOutput")
    taps = {}
    if DEBUG:
        for nm, shape in [("d_t0", [128, K, S]), ("d_T2", [128, K, S]),
                          ("d_bt00", [128, SEC]), ("d_eT0", [128, CH * 128]),
                          ("d_cs0", [128, CH, SG, K]), ("d_y0", [128, 128]),
                          ("d_Y2", [128, G, SG, K]), ("d_s1", [128, KQ, D]),
                          ("d_vsb0", [128, KQ, D]), ("d_t1", [128, K, S]),
                          ("d_T21", [128, K, S]), ("d_bt20", [128, SEC]),
                          ("d_y20", [128, 128]), ("d_Y22", [128, G, SG, K]),
                          ("d_s2", [128, KQ, D])]:
            taps[nm] = nc.dram_tensor(nm, shape, F32, kind="ExternalOutput")

    with tile.TileContext(nc) as tc, ExitStack() as ctx:
        const = ctx.enter_context(tc.tile_pool(name="const", bufs=1))
        xap = ctx.enter_context(tc.tile_pool(name="xa", bufs=1))
        xbp = ctx.enter_context(tc.tile_pool(name="xb", bufs=1))
        etp = ctx.enter_context(tc.tile_pool(name="et", bufs=2))
        csp = ctx.enter_context(tc.tile_pool(name="cs", bufs=2))
        zp = ctx.enter_context(tc.tile_pool(name="z", bufs=3))
        y2p = ctx.enter_context(tc.tile_pool(name="y2", bufs=1))
        ysp = ctx.enter_context(tc.tile_pool(name="ys", bufs=2))
        sqp = ctx.enter_context(tc.tile_pool(name="sqp", bufs=1))
        t2p = ctx.enter_context(tc.tile_pool(name="t2", bufs=2))
        vt2p = ctx.enter_context(tc.tile_pool(name="vt2", bufs=1))
        # PSUM: every buf is one full 2KB bank; exactly 8 in total.
        ps_t = ctx.enter_context(tc.tile_pool(name="ps_t", bufs=1, space="PSUM"))
        ps_e = ctx.enter_context(tc.tile_pool(name="ps_e", bufs=2, space="PSUM"))
        ps_bt = ctx.enter_context(tc.tile_pool(name="ps_bt", bufs=2, space="PSUM"))
        ps_y = ctx.enter_context(tc.tile_pool(name="ps_y", bufs=1, space="PSUM"))
        ps_s = ctx.enter_context(tc.tile_pool(name="ps_s", bufs=1, space="PSUM"))
        ps_vt = ctx.enter_context(tc.tile_pool(name="ps_vt", bufs=1, space="PSUM"))

        # ---- constants, host-precomputed, loaded at top scheduler priority
        with tc.high_priority():
            wr = const.tile([128, K, D], F32)
            nc.sync.dma_start(out=wr, in_=wr_d[:])
            y0t = const.tile([128, S], F32)
            nc.sync.dma_start(out=y0t, in_=y0_d[:])
            wt = const.tile([32, K, 128], BF16)
            nc.sync.dma_start(out=wt, in_=wt_d[:])
            idz = const.tile([128, 132], BF16)
            nc.sync.dma_start(out=idz, in_=idz_d[:])
            id128f = const.tile([128, 128], F32)
            nc.sync.dma_start(out=id128f, in_=idf_d[:])
            id16 = const.tile([16, 16], F32)
            nc.sync.dma_start(out=id16, in_=id16_d[:])

        # ---- x loads, interleaved xb/xa per sample across both HWDGE queues
        dma_engines = [nc.sync, nc.scalar]
        xb_s = []
        xa_s = []
        for s in range(S):
            tb = xbp.tile([128, CH * 128], BF16, tag=f"xb{s}")
            dma_engines[s % 2].dma_start(out=tb, in_=xb_d[:, s])
            xb_s.append(tb)
            ta = xap.tile([128, CH, 128], BF16, tag=f"xa{s}")
            dma_engines[(s + 1) % 2].dma_start(out=ta, in_=xa_d[:, s])
            xa_s.append(ta)

        # ---- iteration 0: s from host-precomputed uniform-softmax y0
        s_ps = ps_s.tile([128, KQ, D], F32, tag="s")
        # strips only fill 16 of each 32 partitions; init the rest for squash
        nc.vector.memset(s_ps, 0.0)
        for k in range(K):
            t, kq = k % 4, k // 4
            nc.tensor.matmul(s_ps[32 * t:32 * t + S, kq, :], y0t,
                             wr[:, k, :], start=True, stop=True,
                             tile_position=(0, 32 * t),
                             skip_group_check=True)
        t_ps = ps_t.tile([128, K, S], F32, tag="t")

        def t_pass(vsb, first, T2_prev=None):
            """vt transposes + t-matmul; T2 accumulates across passes in bf16.

            The strip layout is first collected to partition base 0 via
            SBUF->SBUF DMAs: PE transposes at non-zero row tile_positions
            hang the device (NRT_EXEC_UNIT_UNRECOVERABLE) unless used as a
            single ascending sequence, so all transposes run at (0, 0).
            """
            vsb0 = sqp.tile([16, 4, KQ, D], F32, tag="vsb0")
            with tc.high_priority():
                for t in range(4):
                    nc.sync.dma_start(out=vsb0[:, t],
                                      in_=vsb[32 * t:32 * t + S])
            vt_ps = ps_vt.tile([32, K, S], F32, tag="vt")
            for t in range(4):
                for kq in range(KQ):
                    k = 4 * kq + t
                    nc.tensor.transpose(out=vt_ps[:, k, :],
                                        in_=vsb0[:, t, kq, :],
                                        identity=id16)
            T2 = t2p.tile([128, K, S], BF16, tag="T2")
            if STAGE == 21:
                nc.vector.memset(T2, 0.0)
                return T2
            vt2 = vt2p.tile([32, K, S], BF16, tag="vt2")
            nc.vector.tensor_copy(out=vt2, in_=vt_ps)
            for k in range(K):
                nc.tensor.matmul(t_ps[:, k, :], wt[:, k, :], vt2[:, k, :],
                                 start=True, stop=True)
            if STAGE == 22:
                nc.vector.memset(T2, 0.0)
                return T2
            if first:
                nc.vector.tensor_copy(out=T2, in_=t_ps)
            else:
                nc.vector.tensor_tensor(out=T2, in0=t_ps, in1=T2_prev, op=ADD)
            if DEBUG and not first:
                d_t1sb = const.tile([128, K, S], F32, name="d_t1sb")
                nc.vector.tensor_copy(out=d_t1sb, in_=t_ps)
                nc.sync.dma_start(out=taps["d_t1"][:], in_=d_t1sb)
                d_T21sb = const.tile([128, K, S], F32, name="d_T21sb")
                nc.vector.tensor_copy(out=d_T21sb, in_=T2)
                nc.sync.dma_start(out=taps["d_T21"][:], in_=d_T21sb)
            return T2

        vsb = _squash(nc, sqp, s_ps)
        if DEBUG:
            nc.sync.dma_start(out=taps["d_vsb0"][:], in_=vsb)
        def emit_out(vsb):
            vq = v_d[:].rearrange("s (kq four) d -> four s kq d", four=4)
            with tc.high_priority():
                for t in range(4):
                    eng = nc.sync if t % 2 == 0 else nc.scalar
                    eng.dma_start(out=vq[t], in_=vsb[32 * t:32 * t + S])
        if STAGE == 1:
            emit_out(vsb)
        T2 = t_pass(vsb, first=True) if (STAGE >= 2 or STAGE in (21, 22)) else None
        if DEBUG:
            d_t0sb = const.tile([128, K, S], F32, name="d_t0sb")
            nc.vector.tensor_copy(out=d_t0sb, in_=t_ps)
            nc.sync.dma_start(out=taps["d_t0"][:], in_=d_t0sb)
            d_T2sb = const.tile([128, K, S], F32, name="d_T2sb")
            nc.vector.tensor_copy(out=d_T2sb, in_=T2)
            nc.sync.dma_start(out=taps["d_T2"][:], in_=d_T2sb)

        if STAGE in (2, 21, 22):
            emit_out(vsb)

        for p in ((1, 2) if STAGE >= 6 else ((1,) if (STAGE >= 3 and STAGE not in (21, 22)) else ())):
            eT_g = [None] * G
            bt_gs = [[None] * NSEC for _ in range(G)]

            def emit_b(g):
                eT = etp.tile([128, CH * 128], BF16, tag="eT")
                eT_g[g] = eT
                for sec in range(NSEC):
                    bt = ps_bt.tile([128, SEC], F32, tag="bt")
                    bt_gs[g][sec] = bt
                    for t in range(4):
                        s = SG * g + t
                        nc.tensor.matmul(bt[32 * t:32 * t + 32, :],
                                         T2[:, :, s],
                                         xb_s[s][:, sec * SEC:(sec + 1) * SEC],
                                         start=True, stop=True,
                                         tile_position=(0, 32 * t))
                    nc.scalar.activation(out=eT[:, sec * SEC:(sec + 1) * SEC],
                                         in_=bt, func=EXP)
                    if DEBUG and g == 0 and sec == 0:
                        d_btsb = const.tile([128, SEC], F32,
                                            name="d_btsb", tag=f"d_btsb{p}")
                        nc.vector.tensor_copy(out=d_btsb, in_=bt)
                        nm = "d_bt00" if p == 1 else "d_bt20"
                        nc.sync.dma_start(out=taps[nm][:], in_=d_btsb)

            y_ps = ps_y.tile([128, G, 128], F32, tag="y")

            def emit_smy(g):
                eT = eT_g[g]
                # transpose fused with Z: identity is [I128 | block-ones(4)],
                # so cols 128:132 of each transposed chunk hold the per-sample
                # k-sums (softmax denominators) computed by the PE for free.
                e_t = ps_e.tile([128, 3, 132], F32, tag="e")
                cs = csp.tile([128, CH, SG, K], BF16, tag="cs")
                for ic0 in range(0, CH, 3):
                    nsl = min(3, CH - ic0)
                    for ic in range(ic0, ic0 + nsl):
                        nc.tensor.matmul(
                            e_t[:, ic % 3],
                            eT[:, ic * 128:(ic + 1) * 128],
                            idz, start=True, stop=True)
                    sl0 = ic0 % 3
                    esl = e_t[:, sl0:sl0 + nsl, 0:128].rearrange(
                        "p n (sg k) -> p n sg k", sg=SG)
                    r = zp.tile([128, 3, SG], F32, tag="r")
                    nc.vector.reciprocal(out=r[:, 0:nsl],
                                         in_=e_t[:, sl0:sl0 + nsl, 128:132])
                    nc.vector.tensor_tensor(
                        out=cs[:, ic0:ic0 + nsl], in0=esl,
                        in1=r[:, 0:nsl].unsqueeze(-1).broadcast_to(
                            [128, nsl, SG, K]),
                        op=MULT)
                for ic in range(CH):
                    for t in range(4):
                        nc.tensor.matmul(y_ps[32 * t:32 * t + 32, g, :],
                                         cs[:, ic, t, :],
                                         xa_s[SG * g + t][:, ic, :],
                                         start=(ic == 0), stop=(ic == CH - 1),
                                         tile_position=(0, 32 * t),
                                         skip_group_check=True)
                ysb = ysp.tile([128, 128], F32, tag="ysb")
                nc.vector.tensor_copy(out=ysb, in_=y_ps[:, g, :])
                nc.tensor.transpose(out=y_ps[:, g, :], in_=ysb,
                                    identity=id128f)
                nc.vector.tensor_copy(out=Y2[:, g].rearrange("p sg k -> p (sg k)"),
                                      in_=y_ps[:, g, :])
                if DEBUG and p == 2 and g == 0:
                    d_y2sb = const.tile([128, 128], F32, name="d_y2sb")
                    nc.vector.tensor_copy(out=d_y2sb, in_=y_ps[:, 0, :])
                    nc.sync.dma_start(out=taps["d_y20"][:], in_=d_y2sb)
                if DEBUG and p == 1 and g == 0:
                    d_eTsb = const.tile([128, CH * 128], F32, name="d_eTsb")
                    nc.vector.tensor_copy(out=d_eTsb, in_=eT)
                    nc.sync.dma_start(out=taps["d_eT0"][:], in_=d_eTsb)
                    d_cssb = const.tile([128, CH, SG, K], F32, name="d_cssb")
                    nc.vector.tensor_copy(out=d_cssb, in_=cs)
                    nc.sync.dma_start(out=taps["d_cs0"][:], in_=d_cssb)
                    d_ysb = const.tile([128, 128], F32, name="d_ysb")
                    nc.vector.tensor_copy(out=d_ysb, in_=y_ps[:, 0, :])
                    nc.sync.dma_start(out=taps["d_y0"][:], in_=d_ysb)

            Y2 = y2p.tile([128, G, SG, K], F32, tag="Y2")
            # software-pipelined emission: PE never waits on ACT exp
            if STAGE == 3:
                emit_b(0); emit_b(1); emit_b(2); emit_b(3)
                emit_out(vsb)
                break
            emit_b(0)
            emit_b(1)
            emit_smy(0)
            emit_b(2)
            emit_smy(1)
            emit_b(3)
            emit_smy(2)
            emit_smy(3)
            if STAGE == 4:
                emit_out(vsb)
                break

            # s-matmul, column-tiled: capsule k -> strip t=k%4, row kq=k//4
            if STAGE == 4:
                break
            s_ps = ps_s.tile([128, KQ, D], F32, tag="s")
            nc.vector.memset(s_ps, 0.0)
            for k in range(K):
                t, kq = k % 4, k // 4
                nc.tensor.matmul(s_ps[32 * t:32 * t + S, kq, :],
                                 Y2[:, :, :, k], wr[:, k, :],
                                 start=True, stop=True,
                                 tile_position=(0, 32 * t),
                                 skip_group_check=True)
            if DEBUG:
                nc.sync.dma_start(out=taps["d_Y2" if p == 1 else "d_Y22"][:],
                                  in_=Y2)
                d_s1sb = const.tile([128, KQ, D], F32,
                                    name="d_s1sb", tag=f"d_s1sb{p}")
                nc.vector.tensor_copy(out=d_s1sb, in_=s_ps)
                nc.sync.dma_start(out=taps["d_s1" if p == 1 else "d_s2"][:],
                                  in_=d_s1sb)
            vsb = _squash(nc, sqp, s_ps)
            if p == 1:
                T2 = t_pass(vsb, first=False, T2_prev=T2)
                if STAGE == 5:
                    emit_out(vsb)
            else:
                emit_out(vsb)

    nc.compile()
    return nc


def _get_program():
    global _PROGRAM
    if _PROGRAM is None:
        _PROGRAM = _build_program()
    return _PROGRAM


def _prep_core_inputs(x_core, wr, wt):
    """x_core: [S, I, J] fp32 -> per-core input map."""
    bf = ml_dtypes.bfloat16
    xa = np.ascontiguousarray(
        x_core.reshape(S, CH, 128, J).transpose(2, 0, 1, 3).astype(bf))  # [128,S,CH,J]
    xb = np.ascontiguousarray(x_core.transpose(2, 0, 1).astype(bf))      # [J,S,I]
    y0 = np.ascontiguousarray((x_core.sum(axis=1) / K).T)                # [J,S] f32
    return {"xa": xa, "xb": xb.reshape(J, S, CH * 128), "wr": wr,
            "wt": wt, "y0": y0,
            "idz": np.concatenate(
                [np.eye(128, dtype=np.float32),
                 np.kron(np.eye(4, dtype=np.float32), np.ones((32, 1), np.float32))],
                axis=1).astype(bf),
            "idf": np.eye(128, dtype=np.float32),
            "id16": np.eye(16, dtype=np.float32)}


def kernel(inputs, W):
    x = np.ascontiguousarray(np.asarray(inputs, dtype=np.float32))
    Wf = np.ascontiguousarray(np.asarray(W, dtype=np.float32))           # [J, K, D]
    wt = np.ascontiguousarray(
        Wf.transpose(2, 1, 0).astype(ml_dtypes.bfloat16))                # [D, K, J]
    nc = _get_program()
    in_maps = [_prep_core_inputs(x[c * S:(c + 1) * S], Wf, wt) for c in range(NCORES)]
    res = run_bass_kernel_spmd(nc, in_maps, list(range(NCORES)))
    return np.concatenate([r["vout"] for r in res.results], axis=0)


# revision 29
# speedup vs baseline: 1.0713x; 1.0713x over previous
"""Trainium2 Bass kernel for CapsuleLayer dynamic routing (B=128, I=1152, J=128, K=32, D=32).

Strategy
--------
Data-parallel over batch: 16 samples per core x 8 cores. u_hat is never
materialized; the routing math is factorized exactly as in the baseline:

    y[s,k,j]  = sum_i c[s,i,k] x[s,i,j]
    s[s,k,d]  = sum_j y[s,k,j] W[j,k,d]
    v         = squash(s)
    t[s,j,k]  = sum_d W[j,k,d] v[s,k,d]
    b[s,i,k]  = sum_j x[s,i,j] (t0+..+t_it)[s,j,k]   (linear in t -> t accumulates
                                                      in PSUM, no b state kept)

PE orientation is flipped vs the baseline: the small per-sample matrices
(c-chunks / t, 32 columns) are the matmul *stationary* (cheap LDWEIGHTS) and x
is the *moving* operand (N=128..384), with 4x column tiling running 4 samples
concurrently in 32-column strips of the PE array. Outputs land transposed
(b^T, y^T); b^T is fixed by fusing softmax's exp (ACT, PSUM->SBUF) with PE
128x128 transposes, y^T by one PE transpose per sample group. Softmax
normalization and squash run on DVE; sqrt is computed as exp(0.5*ln(x)) so ACT
only ever uses one table set (no ACT_TABLE_LOAD thrash). s-matmuls are
column-tiled into a [128, K/4, D] strip layout (sample s of strip t lives at
partition 32t+s, capsule k = 4*kq + t).
"""
import numpy as np
import ml_dtypes
from contextlib import ExitStack

import concourse.bass as bass
import concourse.bacc as bacc_mod
import concourse.mybir as mybir
import concourse.tile as tile
from concourse.bass_utils import run_bass_kernel_spmd
from concourse.masks import make_identity

B, I, J, K, D = 128, 1152, 128, 32, 32
NCORES = 8
S = B // NCORES          # 16 samples per core
CH = I // 128            # 9 chunks of the input-capsule axis
G = 4                    # sample groups per core
SG = S // G              # 4 samples per group (one per 32-col PE strip)
KQ = K // 4              # 8 capsules per strip in the s/v layout
NSEC = 3                 # b^T i-axis sections (384 columns each)
SEC = I // NSEC
NUM_ROUTING = 3
EPS = 1e-7
F32 = mybir.dt.float32
F32R = mybir.dt.float32r
BF16 = mybir.dt.bfloat16
AX = mybir.AxisListType.X
ADD = mybir.AluOpType.add
MULT = mybir.AluOpType.mult
EXP = mybir.ActivationFunctionType.Exp
U32 = mybir.dt.uint32
A_LSR = mybir.AluOpType.logical_shift_right
A_SUB = mybir.AluOpType.subtract

_PROGRAM = None
DEBUG = False
STAGE = 6  # debug truncation: 1=it0, 2=+t0, 3=+p1 b/exp, 4=+p1 smy, 5=+p1 s/t, 6=full


def _squash(nc, sqp, s_ps, act_square=True):
    """vsb = squash(s_ps) along d. s_ps: [128, KQ, D] PSUM (strip layout).

    Pure-DVE: rsqrt via the 0x5F3759DF bit trick (fp-domain magic subtract,
    exact enough for a seed) + 2 Newton iterations. No ACT involvement, so
    the scalar engine only ever loads the softmax Exp table set once.
    scale = ss * rsqrt(ss) / (1 + ss); the reference's eps only matters at
    ss ~ eps where |v| < 1e-4 absolute, far under tolerance.
    """
    sq = sqp.tile([128, KQ, D], F32, tag="sq")
    if act_square:
        nc.scalar.activation(out=sq, in_=s_ps,
                             func=mybir.ActivationFunctionType.Square)
    else:  # keep it0 off the scalar queue (it is busy with input DMAs)
        s_sb = sqp.tile([128, KQ, D], F32, tag="s_sb")
        nc.vector.tensor_copy(out=s_sb, in_=s_ps)
        nc.vector.tensor_tensor(out=sq, in0=s_ps, in1=s_sb, op=MULT)
    ss = sqp.tile([128, KQ], F32, tag="ss")
    nc.vector.tensor_reduce(out=ss, in_=sq, axis=AX, op=ADD)
    # floor ss so the Newton iterations can't overflow (ss=0 -> seed^2 = inf)
    nc.vector.tensor_scalar(out=ss, in0=ss, scalar1=1e-12, scalar2=None,
                            op0=mybir.AluOpType.max)
    ib = sqp.tile([128, KQ], U32, tag="ib")
    nc.vector.tensor_scalar(out=ib, in0=ss.bitcast(U32), scalar1=1,
                            scalar2=None, op0=A_LSR)
    nc.vector.tensor_scalar(out=ib, in0=ib, scalar1=float(0x5F3759DF),
                            scalar2=-1.0, op0=A_SUB, op1=MULT)
    y = sqp.tile([128, KQ], F32, tag="yns")
    t1 = sqp.tile([128, KQ], F32, tag="t1ns")
    rs = ib.bitcast(F32)
    for _ in range(2):
        nc.vector.tensor_mul(t1, rs, rs)
        nc.vector.tensor_mul(t1, t1, ss)
        nc.vector.tensor_scalar(out=t1, in0=t1, scalar1=-0.5, scalar2=1.5,
                                op0=MULT, op1=ADD)
        nc.vector.tensor_mul(y, rs, t1)
        rs = y
    den = sqp.tile([128, KQ], F32, tag="den")
    nc.vector.tensor_scalar(out=den, in0=ss, scalar1=1.0, scalar2=None,
                            op0=ADD)
    rden = sqp.tile([128, KQ], F32, tag="rden")
    nc.vector.reciprocal(out=rden, in_=den)
    sc = sqp.tile([128, KQ], F32, tag="sc")
    nc.vector.tensor_mul(sc, ss, y)
    nc.vector.tensor_mul(sc, sc, rden)
    vsb = sqp.tile([128, KQ, D], F32, tag="vsb")
    nc.vector.tensor_tensor(out=vsb, in0=s_ps,
                            in1=sc.unsqueeze(-1).broadcast_to([128, KQ, D]),
                            op=MULT)
    return vsb


def _build_program():
    nc = bacc_mod.Bacc("TRN2", target_bir_lowering=False, debug=False,
                       num_devices=NCORES)
    xa_d = nc.dram_tensor("xa", [128, S, CH, 128], BF16, kind="ExternalInput")
    xb_d = nc.dram_tensor("xb", [128, S, CH * 128], BF16, kind="ExternalInput")
    wr_d = nc.dram_tensor("wr", [128, K, D], F32, kind="ExternalInput")
    wt_d = nc.dram_tensor("wt", [32, K, 128], BF16, kind="ExternalInput")
    y0_d = nc.dram_tensor("y0", [128, S], F32, kind="ExternalInput")
    idz_d = nc.dram_tensor("idz", [128, 132], BF16, kind="ExternalInput")
    idf_d = nc.dram_tensor("idf", [128, 128], F32, kind="ExternalInput")
    id16_d = nc.dram_tensor("id16", [16, 16], F32, kind="ExternalInput")
    v_d = nc.dram_tensor("vout", [S, K, D], F32, kind="ExternalOutput")
    taps = {}
    if DEBUG:
        for nm, shape in [("d_t0", [128, K, S]), ("d_T2", [128, K, S]),
                          ("d_bt00", [128, SEC]), ("d_eT0", [128, CH * 128]),
                          ("d_cs0", [128, CH, SG, K]), ("d_y0", [128, 128]),
                          ("d_Y2", [128, G, SG, K]), ("d_s1", [128, KQ, D]),
                          ("d_vsb0", [128, KQ, D]), ("d_t1", [128, K, S]),
                          ("d_T21", [128, K, S]), ("d_bt20", [128, SEC]),
                          ("d_y20", [128, 128]), ("d_Y22", [128, G, SG, K]),
                          ("d_s2", [128, KQ, D])]:
            taps[nm] = nc.dram_tensor(nm, shape, F32, kind="ExternalOutput")

    with tile.TileContext(nc) as tc, ExitStack() as ctx:
        const = ctx.enter_context(tc.tile_pool(name="const", bufs=1))
        xap = ctx.enter_context(tc.tile_pool(name="xa", bufs=1))
        xbp = ctx.enter_context(tc.tile_pool(name="xb", bufs=1))
        etp = ctx.enter_context(tc.tile_pool(name="et", bufs=2))
        csp = ctx.enter_context(tc.tile_pool(name="cs", bufs=2))
        zp = ctx.enter_context(tc.tile_pool(name="z", bufs=3))
        y2p = ctx.enter_context(tc.tile_pool(name="y2", bufs=1))
        ysp = ctx.enter_context(tc.tile_pool(name="ys", bufs=2))
        sqp = ctx.enter_context(tc.tile_pool(name="sqp", bufs=1))
        t2p = ctx.enter_context(tc.tile_pool(name="t2", bufs=2))
        vt2p = ctx.enter_context(tc.tile_pool(name="vt2", bufs=1))
        # PSUM: every buf is one full 2KB bank; exactly 8 in total.
        ps_t = ctx.enter_context(tc.tile_pool(name="ps_t", bufs=1, space="PSUM"))
        ps_e = ctx.enter_context(tc.tile_pool(name="ps_e", bufs=2, space="PSUM"))
        ps_bt = ctx.enter_context(tc.tile_pool(name="ps_bt", bufs=2, space="PSUM"))
        ps_y = ctx.enter_context(tc.tile_pool(name="ps_y", bufs=1, space="PSUM"))
        ps_s = ctx.enter_context(tc.tile_pool(name="ps_s", bufs=1, space="PSUM"))
        ps_vt = ctx.enter_context(tc.tile_pool(name="ps_vt", bufs=1, space="PSUM"))

        # ---- constants, host-precomputed, loaded at top scheduler priority
        with tc.high_priority():
            wr = const.tile([128, K, D], F32)
            nc.sync.dma_start(out=wr, in_=wr_d[:])
            y0t = const.tile([128, S], F32)
            nc.sync.dma_start(out=y0t, in_=y0_d[:])
            wt = const.tile([32, K, 128], BF16)
            nc.sync.dma_start(out=wt, in_=wt_d[:])
            idz = const.tile([128, 132], BF16)
            nc.sync.dma_start(out=idz, in_=idz_d[:])
            id128f = const.tile([128, 128], F32)
            nc.sync.dma_start(out=id128f, in_=idf_d[:])
            id16 = const.tile([16, 16], F32)
            nc.sync.dma_start(out=id16, in_=id16_d[:])

        # ---- x loads, batched per sample-group across both HWDGE queues
        dma_engines = [nc.sync, nc.scalar]
        xb_g = []
        xa_g = []
        for g in range(G):
            tb = xbp.tile([128, SG, CH * 128], BF16, tag=f"xb{g}")
            dma_engines[g % 2].dma_start(out=tb, in_=xb_d[:, SG * g:SG * g + SG])
            xb_g.append(tb)
            ta = xap.tile([128, SG, CH, 128], BF16, tag=f"xa{g}")
            dma_engines[(g + 1) % 2].dma_start(out=ta,
                                               in_=xa_d[:, SG * g:SG * g + SG])
            xa_g.append(ta)

        # ---- iteration 0: s from host-precomputed uniform-softmax y0
        s_ps = ps_s.tile([128, KQ, D], F32, tag="s")
        # strips only fill 16 of each 32 partitions; init the rest for squash
        nc.vector.memset(s_ps, 0.0)
        for k in range(K):
            t, kq = k % 4, k // 4
            nc.tensor.matmul(s_ps[32 * t:32 * t + S, kq, :], y0t,
                             wr[:, k, :], start=True, stop=True,
                             tile_position=(0, 32 * t),
                             skip_group_check=True)
        t_ps = ps_t.tile([128, K, S], F32, tag="t")

        def t_pass(vsb, first, T2_prev=None):
            """vt transposes + t-matmul; T2 accumulates across passes in bf16.

            The strip layout is first collected to partition base 0 via
            SBUF->SBUF DMAs: PE transposes at non-zero row tile_positions
            hang the device (NRT_EXEC_UNIT_UNRECOVERABLE) unless used as a
            single ascending sequence, so all transposes run at (0, 0).
            """
            vsb0 = sqp.tile([16, 4, KQ, D], F32, tag="vsb0")
            with tc.high_priority():
                for t in range(4):
                    nc.sync.dma_start(out=vsb0[:, t],
                                      in_=vsb[32 * t:32 * t + S])
            vt_ps = ps_vt.tile([32, K, S], F32, tag="vt")
            for t in range(4):
                for kq in range(KQ):
                    k = 4 * kq + t
                    nc.tensor.transpose(out=vt_ps[:, k, :],
                                        in_=vsb0[:, t, kq, :],
                                        identity=id16)
            T2 = t2p.tile([128, K, S], BF16, tag="T2")
            if STAGE == 21:
                nc.vector.memset(T2, 0.0)
                return T2
            vt2 = vt2p.tile([32, K, S], BF16, tag="vt2")
            nc.vector.tensor_copy(out=vt2, in_=vt_ps)
            for k in range(K):
                nc.tensor.matmul(t_ps[:, k, :], wt[:, k, :], vt2[:, k, :],
                                 start=True, stop=True)
            if STAGE == 22:
                nc.vector.memset(T2, 0.0)
                return T2
            if first:
                nc.vector.tensor_copy(out=T2, in_=t_ps)
            else:
                nc.vector.tensor_tensor(out=T2, in0=t_ps, in1=T2_prev, op=ADD)
            if DEBUG and not first:
                d_t1sb = const.tile([128, K, S], F32, name="d_t1sb")
                nc.vector.tensor_copy(out=d_t1sb, in_=t_ps)
                nc.sync.dma_start(out=taps["d_t1"][:], in_=d_t1sb)
                d_T21sb = const.tile([128, K, S], F32, name="d_T21sb")
                nc.vector.tensor_copy(out=d_T21sb, in_=T2)
                nc.sync.dma_start(out=taps["d_T21"][:], in_=d_T21sb)
            return T2

        vsb = _squash(nc, sqp, s_ps, act_square=False)
        if DEBUG:
            nc.sync.dma_start(out=taps["d_vsb0"][:], in_=vsb)
        def emit_out(vsb):
            vq = v_d[:].rearrange("s (kq four) d -> four s kq d", four=4)
            with tc.high_priority():
                for t in range(4):
                    eng = nc.sync if t % 2 == 0 else nc.scalar
                    eng.dma_start(out=vq[t], in_=vsb[32 * t:32 * t + S])
        if STAGE == 1:
            emit_out(vsb)
        T2 = t_pass(vsb, first=True) if (STAGE >= 2 or STAGE in (21, 22)) else None
        if DEBUG:
            d_t0sb = const.tile([128, K, S], F32, name="d_t0sb")
            nc.vector.tensor_copy(out=d_t0sb, in_=t_ps)
            nc.sync.dma_start(out=taps["d_t0"][:], in_=d_t0sb)
            d_T2sb = const.tile([128, K, S], F32, name="d_T2sb")
            nc.vector.tensor_copy(out=d_T2sb, in_=T2)
            nc.sync.dma_start(out=taps["d_T2"][:], in_=d_T2sb)

        if STAGE in (2, 21, 22):
            emit_out(vsb)

        for p in ((1, 2) if STAGE >= 6 else ((1,) if (STAGE >= 3 and STAGE not in (21, 22)) else ())):
            eT_g = [None] * G
            bt_gs = [[None] * NSEC for _ in range(G)]

            def emit_b(g):
                eT = etp.tile([128, CH * 128], BF16, tag="eT")
                eT_g[g] = eT
                for sec in range(NSEC):
                    bt = ps_bt.tile([128, SEC], F32, tag="bt")
                    bt_gs[g][sec] = bt
                    for t in range(4):
                        s = SG * g + t
                        nc.tensor.matmul(bt[32 * t:32 * t + 32, :],
                                         T2[:, :, s],
                                         xb_g[g][:, t, sec * SEC:(sec + 1) * SEC],
                                         start=True, stop=True,
                                         tile_position=(0, 32 * t))
                    nc.scalar.activation(out=eT[:, sec * SEC:(sec + 1) * SEC],
                                         in_=bt, func=EXP)
                    if DEBUG and g == 0 and sec == 0:
                        d_btsb = const.tile([128, SEC], F32,
                                            name="d_btsb", tag=f"d_btsb{p}")
                        nc.vector.tensor_copy(out=d_btsb, in_=bt)
                        nm = "d_bt00" if p == 1 else "d_bt20"
                        nc.sync.dma_start(out=taps[nm][:], in_=d_btsb)

            y_ps = ps_y.tile([128, G, 128], F32, tag="y")

            def emit_smy(g):
                eT = eT_g[g]
                # transpose fused with Z: identity is [I128 | block-ones(4)],
                # so cols 128:132 of each transposed chunk hold the per-sample
                # k-sums (softmax denominators) computed by the PE for free.
                e_t = ps_e.tile([128, 3, 132], F32, tag="e")
                cs = csp.tile([128, CH, SG, K], BF16, tag="cs")
                for ic0 in range(0, CH, 3):
                    nsl = min(3, CH - ic0)
                    for ic in range(ic0, ic0 + nsl):
                        nc.tensor.matmul(
                            e_t[:, ic % 3],
                            eT[:, ic * 128:(ic + 1) * 128],
                            idz, start=True, stop=True)
                    sl0 = ic0 % 3
                    esl = e_t[:, sl0:sl0 + nsl, 0:128].rearrange(
                        "p n (sg k) -> p n sg k", sg=SG)
                    r = zp.tile([128, 3, SG], F32, tag="r")
                    nc.vector.reciprocal(out=r[:, 0:nsl],
                                         in_=e_t[:, sl0:sl0 + nsl, 128:132])
                    nc.vector.tensor_tensor(
                        out=cs[:, ic0:ic0 + nsl], in0=esl,
                        in1=r[:, 0:nsl].unsqueeze(-1).broadcast_to(
                            [128, nsl, SG, K]),
                        op=MULT)
                for ic in range(CH):
                    for t in range(4):
                        nc.tensor.matmul(y_ps[32 * t:32 * t + 32, g, :],
                                         cs[:, ic, t, :],
                                         xa_g[g][:, t, ic, :],
                                         start=(ic == 0), stop=(ic == CH - 1),
                                         tile_position=(0, 32 * t),
                                         skip_group_check=True)
                ysb = ysp.tile([128, 128], F32, tag="ysb")
                nc.vector.tensor_copy(out=ysb, in_=y_ps[:, g, :])
                nc.tensor.transpose(out=y_ps[:, g, :], in_=ysb,
                                    identity=id128f)
                nc.vector.tensor_copy(out=Y2[:, g].rearrange("p sg k -> p (sg k)"),
                                      in_=y_ps[:, g, :])
                if DEBUG and p == 2 and g == 0:
                    d_y2sb = const.tile([128, 128], F32, name="d_y2sb")
                    nc.vector.tensor_copy(out=d_y2sb, in_=y_ps[:, 0, :])
                    nc.sync.dma_start(out=taps["d_y20"][:], in_=d_y2sb)
                if DEBUG and p == 1 and g == 0:
                    d_eTsb = const.tile([128, CH * 128], F32, name="d_eTsb")
                    nc.vector.tensor_copy(out=d_eTsb, in_=eT)
                    nc.sync.dma_start(out=taps["d_eT0"][:], in_=d_eTsb)
                    d_cssb = const.tile([128, CH, SG, K], F32, name="d_cssb")
                    nc.vector.tensor_copy(out=d_cssb, in_=cs)
                    nc.sync.dma_start(out=taps["d_cs0"][:], in_=d_cssb)
                    d_ysb = const.tile([128, 128], F32, name="d_ysb")
                    nc.vector.tensor_copy(out=d_ysb, in_=y_ps[:, 0, :])
                    nc.sync.dma_start(out=taps["d_y0"][:], in_=d_ysb)

            Y2 = y2p.tile([128, G, SG, K], F32, tag="Y2")
            # software-pipelined emission: PE never waits on ACT exp
            if STAGE == 3:
                emit_b(0); emit_b(1); emit_b(2); emit_b(3)
                emit_out(vsb)
                break
            emit_b(0)
            emit_b(1)
            emit_smy(0)
            emit_b(2)
            emit_smy(1)
            emit_b(3)
            emit_smy(2)
            emit_smy(3)
            if STAGE == 4:
                emit_out(vsb)
                break

            # s-matmul, column-tiled: capsule k -> strip t=k%4, row kq=k//4
            if STAGE == 4:
                break
            s_ps = ps_s.tile([128, KQ, D], F32, tag="s")
            nc.vector.memset(s_ps, 0.0)
            for k in range(K):
                t, kq = k % 4, k // 4
                nc.tensor.matmul(s_ps[32 * t:32 * t + S, kq, :],
                                 Y2[:, :, :, k], wr[:, k, :],
                                 start=True, stop=True,
                                 tile_position=(0, 32 * t),
                                 skip_group_check=True)
            if DEBUG:
                nc.sync.dma_start(out=taps["d_Y2" if p == 1 else "d_Y22"][:],
                                  in_=Y2)
                d_s1sb = const.tile([128, KQ, D], F32,
                                    name="d_s1sb", tag=f"d_s1sb{p}")
                nc.vector.tensor_copy(out=d_s1sb, in_=s_ps)
                nc.sync.dma_start(out=taps["d_s1" if p == 1 else "d_s2"][:],
                                  in_=d_s1sb)
            vsb = _squash(nc, sqp, s_ps)
            if p == 1:
                T2 = t_pass(vsb, first=False, T2_prev=T2)
                if STAGE == 5:
                    emit_out(vsb)
            else:
                emit_out(vsb)

    nc.compile()
    return nc


def _get_program():
    global _PROGRAM
    if _PROGRAM is None:
        _PROGRAM = _build_program()
    return _PROGRAM


def _prep_core_inputs(x_core, wr, wt):
    """x_core: [S, I, J] fp32 -> per-core input map."""
    bf = ml_dtypes.bfloat16
    xa = np.ascontiguousarray(
        x_core.reshape(S, CH, 128, J).transpose(2, 0, 1, 3).astype(bf))  # [128,S,CH,J]
    xb = np.ascontiguousarray(x_core.transpose(2, 0, 1).astype(bf))      # [J,S,I]
    y0 = np.ascontiguousarray((x_core.sum(axis=1) / K).T)                # [J,S] f32
    return {"xa": xa, "xb": xb.reshape(J, S, CH * 128), "wr": wr,
            "wt": wt, "y0": y0,
            "idz": np.concatenate(
                [np.eye(128, dtype=np.float32),
                 np.kron(np.eye(4, dtype=np.float32), np.ones((32, 1), np.float32))],
                axis=1).astype(bf),
            "idf": np.eye(128, dtype=np.float32),
            "id16": np.eye(16, dtype=np.float32)}


def kernel(inputs, W):
    x = np.ascontiguousarray(np.asarray(inputs, dtype=np.float32))
    Wf = np.ascontiguousarray(np.asarray(W, dtype=np.float32))           # [J, K, D]
    wt = np.ascontiguousarray(
        Wf.transpose(2, 1, 0).astype(ml_dtypes.bfloat16))                # [D, K, J]
    nc = _get_program()
    in_maps = [_prep_core_inputs(x[c * S:(c + 1) * S], Wf, wt) for c in range(NCORES)]
    res = run_bass_kernel_spmd(nc, in_maps, list(range(NCORES)))
    return np.concatenate([r["vout"] for r in res.results], axis=0)


# revision 30
# speedup vs baseline: 1.1011x; 1.0278x over previous
"""Trainium2 Bass kernel for CapsuleLayer dynamic routing (B=128, I=1152, J=128, K=32, D=32).

Strategy
--------
Data-parallel over batch: 16 samples per core x 8 cores. u_hat is never
materialized; the routing math is factorized exactly as in the baseline:

    y[s,k,j]  = sum_i c[s,i,k] x[s,i,j]
    s[s,k,d]  = sum_j y[s,k,j] W[j,k,d]
    v         = squash(s)
    t[s,j,k]  = sum_d W[j,k,d] v[s,k,d]
    b[s,i,k]  = sum_j x[s,i,j] (t0+..+t_it)[s,j,k]   (linear in t -> t accumulates
                                                      in PSUM, no b state kept)

PE orientation is flipped vs the baseline: the small per-sample matrices
(c-chunks / t, 32 columns) are the matmul *stationary* (cheap LDWEIGHTS) and x
is the *moving* operand (N=128..384), with 4x column tiling running 4 samples
concurrently in 32-column strips of the PE array. Outputs land transposed
(b^T, y^T); b^T is fixed by fusing softmax's exp (ACT, PSUM->SBUF) with PE
128x128 transposes, y^T by one PE transpose per sample group. Softmax
normalization and squash run on DVE; sqrt is computed as exp(0.5*ln(x)) so ACT
only ever uses one table set (no ACT_TABLE_LOAD thrash). s-matmuls are
column-tiled into a [128, K/4, D] strip layout (sample s of strip t lives at
partition 32t+s, capsule k = 4*kq + t).
"""
import numpy as np
import ml_dtypes
from contextlib import ExitStack

import concourse.bass as bass
import concourse.bacc as bacc_mod
import concourse.mybir as mybir
import concourse.tile as tile
from concourse.bass_utils import run_bass_kernel_spmd
from concourse.masks import make_identity

B, I, J, K, D = 128, 1152, 128, 32, 32
NCORES = 8
S = B // NCORES          # 16 samples per core
CH = I // 128            # 9 chunks of the input-capsule axis
G = 4                    # sample groups per core
SG = S // G              # 4 samples per group (one per 32-col PE strip)
KQ = K // 4              # 8 capsules per strip in the s/v layout
NSEC = 3                 # b^T i-axis sections (384 columns each)
SEC = I // NSEC
NUM_ROUTING = 3
EPS = 1e-7
F32 = mybir.dt.float32
F32R = mybir.dt.float32r
BF16 = mybir.dt.bfloat16
AX = mybir.AxisListType.X
ADD = mybir.AluOpType.add
MULT = mybir.AluOpType.mult
EXP = mybir.ActivationFunctionType.Exp
U32 = mybir.dt.uint32
A_LSR = mybir.AluOpType.logical_shift_right
A_SUB = mybir.AluOpType.subtract

_PROGRAM = None
DEBUG = False
STAGE = 6  # debug truncation: 1=it0, 2=+t0, 3=+p1 b/exp, 4=+p1 smy, 5=+p1 s/t, 6=full


def _squash(nc, sqp, s_ps, act_square=True):
    """vsb = squash(s_ps) along d. s_ps: [128, KQ, D] PSUM (strip layout).

    Pure-DVE: rsqrt via the 0x5F3759DF bit trick (fp-domain magic subtract,
    exact enough for a seed) + 2 Newton iterations. No ACT involvement, so
    the scalar engine only ever loads the softmax Exp table set once.
    scale = ss * rsqrt(ss) / (1 + ss); the reference's eps only matters at
    ss ~ eps where |v| < 1e-4 absolute, far under tolerance.
    """
    sq = sqp.tile([128, KQ, D], F32, tag="sq")
    if act_square:
        nc.scalar.activation(out=sq, in_=s_ps,
                             func=mybir.ActivationFunctionType.Square)
    else:  # keep it0 off the scalar queue (it is busy with input DMAs)
        s_sb = sqp.tile([128, KQ, D], F32, tag="s_sb")
        nc.vector.tensor_copy(out=s_sb, in_=s_ps)
        nc.vector.tensor_tensor(out=sq, in0=s_ps, in1=s_sb, op=MULT)
    ss = sqp.tile([128, KQ], F32, tag="ss")
    nc.vector.tensor_reduce(out=ss, in_=sq, axis=AX, op=ADD)
    # floor ss so the Newton iterations can't overflow (ss=0 -> seed^2 = inf)
    nc.vector.tensor_scalar(out=ss, in0=ss, scalar1=1e-12, scalar2=None,
                            op0=mybir.AluOpType.max)
    ib = sqp.tile([128, KQ], U32, tag="ib")
    nc.vector.tensor_scalar(out=ib, in0=ss.bitcast(U32), scalar1=1,
                            scalar2=None, op0=A_LSR)
    nc.vector.tensor_scalar(out=ib, in0=ib, scalar1=float(0x5F3759DF),
                            scalar2=-1.0, op0=A_SUB, op1=MULT)
    y = sqp.tile([128, KQ], F32, tag="yns")
    t1 = sqp.tile([128, KQ], F32, tag="t1ns")
    rs = ib.bitcast(F32)
    for _ in range(2):
        nc.vector.tensor_mul(t1, rs, rs)
        nc.vector.tensor_mul(t1, t1, ss)
        nc.vector.tensor_scalar(out=t1, in0=t1, scalar1=-0.5, scalar2=1.5,
                                op0=MULT, op1=ADD)
        nc.vector.tensor_mul(y, rs, t1)
        rs = y
    den = sqp.tile([128, KQ], F32, tag="den")
    nc.vector.tensor_scalar(out=den, in0=ss, scalar1=1.0, scalar2=None,
                            op0=ADD)
    rden = sqp.tile([128, KQ], F32, tag="rden")
    nc.vector.reciprocal(out=rden, in_=den)
    sc = sqp.tile([128, KQ], F32, tag="sc")
    nc.vector.tensor_mul(sc, ss, y)
    nc.vector.tensor_mul(sc, sc, rden)
    vsb = sqp.tile([128, KQ, D], F32, tag="vsb")
    nc.vector.tensor_tensor(out=vsb, in0=s_ps,
                            in1=sc.unsqueeze(-1).broadcast_to([128, KQ, D]),
                            op=MULT)
    return vsb


def _build_program():
    nc = bacc_mod.Bacc("TRN2", target_bir_lowering=False, debug=False,
                       num_devices=NCORES)
    xa_d = nc.dram_tensor("xa", [128, S, CH, 128], BF16, kind="ExternalInput")
    xb_d = nc.dram_tensor("xb", [128, S, CH * 128], BF16, kind="ExternalInput")
    wr_d = nc.dram_tensor("wr", [128, K, D], F32, kind="ExternalInput")
    wt_d = nc.dram_tensor("wt", [32, K, 128], BF16, kind="ExternalInput")
    y0_d = nc.dram_tensor("y0", [128, S], F32, kind="ExternalInput")
    idz_d = nc.dram_tensor("idz", [128, 132], BF16, kind="ExternalInput")
    idf_d = nc.dram_tensor("idf", [128, 128], F32, kind="ExternalInput")
    id16_d = nc.dram_tensor("id16", [16, 16], F32, kind="ExternalInput")
    v_d = nc.dram_tensor("vout", [S, K, D], F32, kind="ExternalOutput")
    taps = {}
    if DEBUG:
        for nm, shape in [("d_t0", [128, K, S]), ("d_T2", [128, K, S]),
                          ("d_bt00", [128, SEC]), ("d_eT0", [128, CH * 128]),
                          ("d_cs0", [128, CH, SG, K]), ("d_y0", [128, 128]),
                          ("d_Y2", [128, G, SG, K]), ("d_s1", [128, KQ, D]),
                          ("d_vsb0", [128, KQ, D]), ("d_t1", [128, K, S]),
                          ("d_T21", [128, K, S]), ("d_bt20", [128, SEC]),
                          ("d_y20", [128, 128]), ("d_Y22", [128, G, SG, K]),
                          ("d_s2", [128, KQ, D])]:
            taps[nm] = nc.dram_tensor(nm, shape, F32, kind="ExternalOutput")

    with tile.TileContext(nc) as tc, ExitStack() as ctx:
        const = ctx.enter_context(tc.tile_pool(name="const", bufs=1))
        xap = ctx.enter_context(tc.tile_pool(name="xa", bufs=1))
        xbp = ctx.enter_context(tc.tile_pool(name="xb", bufs=1))
        etp = ctx.enter_context(tc.tile_pool(name="et", bufs=2))
        csp = ctx.enter_context(tc.tile_pool(name="cs", bufs=2))
        zp = ctx.enter_context(tc.tile_pool(name="z", bufs=3))
        y2p = ctx.enter_context(tc.tile_pool(name="y2", bufs=1))
        ysp = ctx.enter_context(tc.tile_pool(name="ys", bufs=2))
        sqp = ctx.enter_context(tc.tile_pool(name="sqp", bufs=1))
        t2p = ctx.enter_context(tc.tile_pool(name="t2", bufs=2))
        vt2p = ctx.enter_context(tc.tile_pool(name="vt2", bufs=1))
        # PSUM: every buf is one full 2KB bank; exactly 8 in total.
        ps_t = ctx.enter_context(tc.tile_pool(name="ps_t", bufs=1, space="PSUM"))
        ps_e = ctx.enter_context(tc.tile_pool(name="ps_e", bufs=2, space="PSUM"))
        ps_bt = ctx.enter_context(tc.tile_pool(name="ps_bt", bufs=3, space="PSUM"))
        ps_y = ctx.enter_context(tc.tile_pool(name="ps_y", bufs=1, space="PSUM"))
        ps_s = ctx.enter_context(tc.tile_pool(name="ps_s", bufs=1, space="PSUM"))

        # ---- constants, host-precomputed, loaded at top scheduler priority
        with tc.high_priority():
            wr = const.tile([128, K, D], F32)
            nc.sync.dma_start(out=wr, in_=wr_d[:])
            y0t = const.tile([128, S], F32)
            nc.sync.dma_start(out=y0t, in_=y0_d[:])
            wt = const.tile([32, K, 128], BF16)
            nc.sync.dma_start(out=wt, in_=wt_d[:])
            idz = const.tile([128, 132], BF16)
            nc.sync.dma_start(out=idz, in_=idz_d[:])
            id128f = const.tile([128, 128], F32)
            nc.sync.dma_start(out=id128f, in_=idf_d[:])
            id16 = const.tile([16, 16], F32)
            nc.sync.dma_start(out=id16, in_=id16_d[:])

        # ---- x loads, batched per sample-group across both HWDGE queues
        dma_engines = [nc.sync, nc.scalar]
        xb_g = []
        xa_g = []
        for g in range(G):
            tb = xbp.tile([128, SG, CH * 128], BF16, tag=f"xb{g}")
            dma_engines[g % 2].dma_start(out=tb, in_=xb_d[:, SG * g:SG * g + SG])
            xb_g.append(tb)
            ta = xap.tile([128, SG, CH, 128], BF16, tag=f"xa{g}")
            dma_engines[(g + 1) % 2].dma_start(out=ta,
                                               in_=xa_d[:, SG * g:SG * g + SG])
            xa_g.append(ta)

        # ---- iteration 0: s from host-precomputed uniform-softmax y0
        s_ps = ps_s.tile([128, KQ, D], F32, tag="s")
        # strips only fill 16 of each 32 partitions; init the rest for squash
        nc.vector.memset(s_ps, 0.0)
        for k in range(K):
            t, kq = k % 4, k // 4
            nc.tensor.matmul(s_ps[32 * t:32 * t + S, kq, :], y0t,
                             wr[:, k, :], start=True, stop=True,
                             tile_position=(0, 32 * t),
                             skip_group_check=True)
        t_ps = ps_t.tile([128, K, S], F32, tag="t")

        def t_pass(vsb, first, T2_prev=None):
            """vt transposes + t-matmul; T2 accumulates across passes in bf16.

            The strip layout is first collected to partition base 0 via
            SBUF->SBUF DMAs: PE transposes at non-zero row tile_positions
            hang the device (NRT_EXEC_UNIT_UNRECOVERABLE) unless used as a
            single ascending sequence, so all transposes run at (0, 0).
            """
            vsb0 = sqp.tile([16, 4, KQ, D], F32, tag="vsb0")
            with tc.high_priority():
                for t in range(4):
                    nc.sync.dma_start(out=vsb0[:, t],
                                      in_=vsb[32 * t:32 * t + S])
            vt_ps = ps_s.tile([32, K, S], F32, tag="s")
            for t in range(4):
                for kq in range(KQ):
                    k = 4 * kq + t
                    nc.tensor.transpose(out=vt_ps[:, k, :],
                                        in_=vsb0[:, t, kq, :],
                                        identity=id16)
            T2 = t2p.tile([128, K, S], BF16, tag="T2")
            if STAGE == 21:
                nc.vector.memset(T2, 0.0)
                return T2
            vt2 = vt2p.tile([32, K, S], BF16, tag="vt2")
            nc.vector.tensor_copy(out=vt2, in_=vt_ps)
            for k in range(K):
                nc.tensor.matmul(t_ps[:, k, :], wt[:, k, :], vt2[:, k, :],
                                 start=True, stop=True)
            if STAGE == 22:
                nc.vector.memset(T2, 0.0)
                return T2
            if first:
                nc.vector.tensor_copy(out=T2, in_=t_ps)
            else:
                nc.vector.tensor_tensor(out=T2, in0=t_ps, in1=T2_prev, op=ADD)
            if DEBUG and not first:
                d_t1sb = const.tile([128, K, S], F32, name="d_t1sb")
                nc.vector.tensor_copy(out=d_t1sb, in_=t_ps)
                nc.sync.dma_start(out=taps["d_t1"][:], in_=d_t1sb)
                d_T21sb = const.tile([128, K, S], F32, name="d_T21sb")
                nc.vector.tensor_copy(out=d_T21sb, in_=T2)
                nc.sync.dma_start(out=taps["d_T21"][:], in_=d_T21sb)
            return T2

        vsb = _squash(nc, sqp, s_ps, act_square=False)
        if DEBUG:
            nc.sync.dma_start(out=taps["d_vsb0"][:], in_=vsb)
        def emit_out(vsb):
            vq = v_d[:].rearrange("s (kq four) d -> four s kq d", four=4)
            with tc.high_priority():
                for t in range(4):
                    eng = nc.sync if t % 2 == 0 else nc.scalar
                    eng.dma_start(out=vq[t], in_=vsb[32 * t:32 * t + S])
        if STAGE == 1:
            emit_out(vsb)
        T2 = t_pass(vsb, first=True) if (STAGE >= 2 or STAGE in (21, 22)) else None
        if DEBUG:
            d_t0sb = const.tile([128, K, S], F32, name="d_t0sb")
            nc.vector.tensor_copy(out=d_t0sb, in_=t_ps)
            nc.sync.dma_start(out=taps["d_t0"][:], in_=d_t0sb)
            d_T2sb = const.tile([128, K, S], F32, name="d_T2sb")
            nc.vector.tensor_copy(out=d_T2sb, in_=T2)
            nc.sync.dma_start(out=taps["d_T2"][:], in_=d_T2sb)

        if STAGE in (2, 21, 22):
            emit_out(vsb)

        for p in ((1, 2) if STAGE >= 6 else ((1,) if (STAGE >= 3 and STAGE not in (21, 22)) else ())):
            eT_g = [None] * G
            bt_gs = [[None] * NSEC for _ in range(G)]

            def emit_b(g):
                eT = etp.tile([128, CH * 128], BF16, tag="eT")
                eT_g[g] = eT
                for sec in range(NSEC):
                    bt = ps_bt.tile([128, SEC], F32, tag="bt")
                    bt_gs[g][sec] = bt
                    for t in range(4):
                        s = SG * g + t
                        nc.tensor.matmul(bt[32 * t:32 * t + 32, :],
                                         T2[:, :, s],
                                         xb_g[g][:, t, sec * SEC:(sec + 1) * SEC],
                                         start=True, stop=True,
                                         tile_position=(0, 32 * t))
                    nc.scalar.activation(out=eT[:, sec * SEC:(sec + 1) * SEC],
                                         in_=bt, func=EXP)
                    if DEBUG and g == 0 and sec == 0:
                        d_btsb = const.tile([128, SEC], F32,
                                            name="d_btsb", tag=f"d_btsb{p}")
                        nc.vector.tensor_copy(out=d_btsb, in_=bt)
                        nm = "d_bt00" if p == 1 else "d_bt20"
                        nc.sync.dma_start(out=taps[nm][:], in_=d_btsb)

            y_ps = ps_y.tile([128, G, 128], F32, tag="y")

            def emit_smy(g):
                eT = eT_g[g]
                # transpose fused with Z: identity is [I128 | block-ones(4)],
                # so cols 128:132 of each transposed chunk hold the per-sample
                # k-sums (softmax denominators) computed by the PE for free.
                e_t = ps_e.tile([128, 3, 132], F32, tag="e")
                cs = csp.tile([128, CH, SG, K], BF16, tag="cs")
                for ic0 in range(0, CH, 3):
                    nsl = min(3, CH - ic0)
                    for ic in range(ic0, ic0 + nsl):
                        nc.tensor.matmul(
                            e_t[:, ic % 3],
                            eT[:, ic * 128:(ic + 1) * 128],
                            idz, start=True, stop=True)
                    sl0 = ic0 % 3
                    esl = e_t[:, sl0:sl0 + nsl, 0:128].rearrange(
                        "p n (sg k) -> p n sg k", sg=SG)
                    r = zp.tile([128, 3, SG], F32, tag="r")
                    nc.vector.reciprocal(out=r[:, 0:nsl],
                                         in_=e_t[:, sl0:sl0 + nsl, 128:132])
                    nc.vector.tensor_tensor(
                        out=cs[:, ic0:ic0 + nsl], in0=esl,
                        in1=r[:, 0:nsl].unsqueeze(-1).broadcast_to(
                            [128, nsl, SG, K]),
                        op=MULT)
                for ic in range(CH):
                    for t in range(4):
                        nc.tensor.matmul(y_ps[32 * t:32 * t + 32, g, :],
                                         cs[:, ic, t, :],
                                         xa_g[g][:, t, ic, :],
                                         start=(ic == 0), stop=(ic == CH - 1),
                                         tile_position=(0, 32 * t),
                                         skip_group_check=True)
                ysb = ysp.tile([128, 128], F32, tag="ysb")
                nc.vector.tensor_copy(out=ysb, in_=y_ps[:, g, :])
                nc.tensor.transpose(out=y_ps[:, g, :], in_=ysb,
                                    identity=id128f)
                nc.vector.tensor_copy(out=Y2[:, g].rearrange("p sg k -> p (sg k)"),
                                      in_=y_ps[:, g, :])
                if DEBUG and p == 2 and g == 0:
                    d_y2sb = const.tile([128, 128], F32, name="d_y2sb")
                    nc.vector.tensor_copy(out=d_y2sb, in_=y_ps[:, 0, :])
                    nc.sync.dma_start(out=taps["d_y20"][:], in_=d_y2sb)
                if DEBUG and p == 1 and g == 0:
                    d_eTsb = const.tile([128, CH * 128], F32, name="d_eTsb")
                    nc.vector.tensor_copy(out=d_eTsb, in_=eT)
                    nc.sync.dma_start(out=taps["d_eT0"][:], in_=d_eTsb)
                    d_cssb = const.tile([128, CH, SG, K], F32, name="d_cssb")
                    nc.vector.tensor_copy(out=d_cssb, in_=cs)
                    nc.sync.dma_start(out=taps["d_cs0"][:], in_=d_cssb)
                    d_ysb = const.tile([128, 128], F32, name="d_ysb")
                    nc.vector.tensor_copy(out=d_ysb, in_=y_ps[:, 0, :])
                    nc.sync.dma_start(out=taps["d_y0"][:], in_=d_ysb)

            Y2 = y2p.tile([128, G, SG, K], F32, tag="Y2")
            # software-pipelined emission: PE never waits on ACT exp
            if STAGE == 3:
                emit_b(0); emit_b(1); emit_b(2); emit_b(3)
                emit_out(vsb)
                break
            emit_b(0)
            emit_b(1)
            emit_smy(0)
            emit_b(2)
            emit_smy(1)
            emit_b(3)
            emit_smy(2)
            emit_smy(3)
            if STAGE == 4:
                emit_out(vsb)
                break

            # s-matmul, column-tiled: capsule k -> strip t=k%4, row kq=k//4
            if STAGE == 4:
                break
            s_ps = ps_s.tile([128, KQ, D], F32, tag="s")
            nc.vector.memset(s_ps, 0.0)
            for k in range(K):
                t, kq = k % 4, k // 4
                nc.tensor.matmul(s_ps[32 * t:32 * t + S, kq, :],
                                 Y2[:, :, :, k], wr[:, k, :],
                                 start=True, stop=True,
                                 tile_position=(0, 32 * t),
                                 skip_group_check=True)
            if DEBUG:
                nc.sync.dma_start(out=taps["d_Y2" if p == 1 else "d_Y22"][:],
                                  in_=Y2)
                d_s1sb = const.tile([128, KQ, D], F32,
                                    name="d_s1sb", tag=f"d_s1sb{p}")
                nc.vector.tensor_copy(out=d_s1sb, in_=s_ps)
                nc.sync.dma_start(out=taps["d_s1" if p == 1 else "d_s2"][:],
                                  in_=d_s1sb)
            vsb = _squash(nc, sqp, s_ps)
            if p == 1:
                T2 = t_pass(vsb, first=False, T2_prev=T2)
                if STAGE == 5:
                    emit_out(vsb)
            else:
                emit_out(vsb)

    nc.compile()
    return nc


def _get_program():
    global _PROGRAM
    if _PROGRAM is None:
        _PROGRAM = _build_program()
    return _PROGRAM


def _prep_core_inputs(x_core, wr, wt):
    """x_core: [S, I, J] fp32 -> per-core input map."""
    bf = ml_dtypes.bfloat16
    xa = np.ascontiguousarray(
        x_core.reshape(S, CH, 128, J).transpose(2, 0, 1, 3).astype(bf))  # [128,S,CH,J]
    xb = np.ascontiguousarray(x_core.transpose(2, 0, 1).astype(bf))      # [J,S,I]
    y0 = np.ascontiguousarray((x_core.sum(axis=1) / K).T)                # [J,S] f32
    return {"xa": xa, "xb": xb.reshape(J, S, CH * 128), "wr": wr,
            "wt": wt, "y0": y0,
            "idz": np.concatenate(
                [np.eye(128, dtype=np.float32),
                 np.kron(np.eye(4, dtype=np.float32), np.ones((32, 1), np.float32))],
                axis=1).astype(bf),
            "idf": np.eye(128, dtype=np.float32),
            "id16": np.eye(16, dtype=np.float32)}


def kernel(inputs, W):
    x = np.ascontiguousarray(np.asarray(inputs, dtype=np.float32))
    Wf = np.ascontiguousarray(np.asarray(W, dtype=np.float32))           # [J, K, D]
    wt = np.ascontiguousarray(
        Wf.transpose(2, 1, 0).astype(ml_dtypes.bfloat16))                # [D, K, J]
    nc = _get_program()
    in_maps = [_prep_core_inputs(x[c * S:(c + 1) * S], Wf, wt) for c in range(NCORES)]
    res = run_bass_kernel_spmd(nc, in_maps, list(range(NCORES)))
    return np.concatenate([r["vout"] for r in res.results], axis=0)


# revision 31
# speedup vs baseline: 1.1056x; 1.0041x over previous
"""Trainium2 Bass kernel for CapsuleLayer dynamic routing (B=128, I=1152, J=128, K=32, D=32).

Strategy
--------
Data-parallel over batch: 16 samples per core x 8 cores. u_hat is never
materialized; the routing math is factorized exactly as in the baseline:

    y[s,k,j]  = sum_i c[s,i,k] x[s,i,j]
    s[s,k,d]  = sum_j y[s,k,j] W[j,k,d]
    v         = squash(s)
    t[s,j,k]  = sum_d W[j,k,d] v[s,k,d]
    b[s,i,k]  = sum_j x[s,i,j] (t0+..+t_it)[s,j,k]   (linear in t -> t accumulates
                                                      in PSUM, no b state kept)

PE orientation is flipped vs the baseline: the small per-sample matrices
(c-chunks / t, 32 columns) are the matmul *stationary* (cheap LDWEIGHTS) and x
is the *moving* operand (N=128..384), with 4x column tiling running 4 samples
concurrently in 32-column strips of the PE array. Outputs land transposed
(b^T, y^T); b^T is fixed by fusing softmax's exp (ACT, PSUM->SBUF) with PE
128x128 transposes, y^T by one PE transpose per sample group. Softmax
normalization and squash run on DVE; sqrt is computed as exp(0.5*ln(x)) so ACT
only ever uses one table set (no ACT_TABLE_LOAD thrash). s-matmuls are
column-tiled into a [128, K/4, D] strip layout (sample s of strip t lives at
partition 32t+s, capsule k = 4*kq + t).
"""
import numpy as np
import ml_dtypes
from contextlib import ExitStack

import concourse.bass as bass
import concourse.bacc as bacc_mod
import concourse.mybir as mybir
import concourse.tile as tile
from concourse.bass_utils import run_bass_kernel_spmd
from concourse.masks import make_identity

B, I, J, K, D = 128, 1152, 128, 32, 32
NCORES = 8
S = B // NCORES          # 16 samples per core
CH = I // 128            # 9 chunks of the input-capsule axis
G = 4                    # sample groups per core
SG = S // G              # 4 samples per group (one per 32-col PE strip)
KQ = K // 4              # 8 capsules per strip in the s/v layout
NSEC = 3                 # b^T i-axis sections (384 columns each)
SEC = I // NSEC
NUM_ROUTING = 3
EPS = 1e-7
F32 = mybir.dt.float32
F32R = mybir.dt.float32r
BF16 = mybir.dt.bfloat16
AX = mybir.AxisListType.X
ADD = mybir.AluOpType.add
MULT = mybir.AluOpType.mult
EXP = mybir.ActivationFunctionType.Exp
U32 = mybir.dt.uint32
A_LSR = mybir.AluOpType.logical_shift_right
A_SUB = mybir.AluOpType.subtract

_PROGRAM = None
DEBUG = False
STAGE = 6  # debug truncation: 1=it0, 2=+t0, 3=+p1 b/exp, 4=+p1 smy, 5=+p1 s/t, 6=full


def _squash(nc, sqp, s_ps, act_square=True):
    """vsb = squash(s_ps) along d. s_ps: [128, KQ, D] PSUM (strip layout).

    Pure-DVE: rsqrt via the 0x5F3759DF bit trick (fp-domain magic subtract,
    exact enough for a seed) + 2 Newton iterations. No ACT involvement, so
    the scalar engine only ever loads the softmax Exp table set once.
    scale = ss * rsqrt(ss) / (1 + ss); the reference's eps only matters at
    ss ~ eps where |v| < 1e-4 absolute, far under tolerance.
    """
    sq = sqp.tile([128, KQ, D], F32, tag="sq")
    if act_square:
        nc.scalar.activation(out=sq, in_=s_ps,
                             func=mybir.ActivationFunctionType.Square)
    else:  # keep it0 off the scalar queue (it is busy with input DMAs)
        s_sb = sqp.tile([128, KQ, D], F32, tag="s_sb")
        nc.vector.tensor_copy(out=s_sb, in_=s_ps)
        nc.vector.tensor_tensor(out=sq, in0=s_ps, in1=s_sb, op=MULT)
    ss = sqp.tile([128, KQ], F32, tag="ss")
    nc.vector.tensor_reduce(out=ss, in_=sq, axis=AX, op=ADD)
    # floor ss so the Newton iterations can't overflow (ss=0 -> seed^2 = inf)
    nc.vector.tensor_scalar(out=ss, in0=ss, scalar1=1e-12, scalar2=None,
                            op0=mybir.AluOpType.max)
    ib = sqp.tile([128, KQ], U32, tag="ib")
    nc.vector.tensor_scalar(out=ib, in0=ss.bitcast(U32), scalar1=1,
                            scalar2=None, op0=A_LSR)
    nc.vector.tensor_scalar(out=ib, in0=ib, scalar1=float(0x5F3759DF),
                            scalar2=-1.0, op0=A_SUB, op1=MULT)
    y = sqp.tile([128, KQ], F32, tag="yns")
    t1 = sqp.tile([128, KQ], F32, tag="t1ns")
    rs = ib.bitcast(F32)
    for _ in range(2):
        nc.vector.tensor_mul(t1, rs, rs)
        nc.vector.tensor_mul(t1, t1, ss)
        nc.vector.tensor_scalar(out=t1, in0=t1, scalar1=-0.5, scalar2=1.5,
                                op0=MULT, op1=ADD)
        nc.vector.tensor_mul(y, rs, t1)
        rs = y
    den = sqp.tile([128, KQ], F32, tag="den")
    nc.vector.tensor_scalar(out=den, in0=ss, scalar1=1.0, scalar2=None,
                            op0=ADD)
    rden = sqp.tile([128, KQ], F32, tag="rden")
    nc.vector.reciprocal(out=rden, in_=den)
    sc = sqp.tile([128, KQ], F32, tag="sc")
    nc.vector.tensor_mul(sc, ss, y)
    nc.vector.tensor_mul(sc, sc, rden)
    vsb = sqp.tile([128, KQ, D], F32, tag="vsb")
    nc.vector.tensor_tensor(out=vsb, in0=s_ps,
                            in1=sc.unsqueeze(-1).broadcast_to([128, KQ, D]),
                            op=MULT)
    return vsb


def _build_program():
    nc = bacc_mod.Bacc("TRN2", target_bir_lowering=False, debug=False,
                       num_devices=NCORES)
    xa_d = nc.dram_tensor("xa", [128, S, CH, 128], BF16, kind="ExternalInput")
    xb_d = nc.dram_tensor("xb", [128, S, CH * 128], BF16, kind="ExternalInput")
    wr_d = nc.dram_tensor("wr", [128, K, D], F32, kind="ExternalInput")
    wt_d = nc.dram_tensor("wt", [32, K, 128], BF16, kind="ExternalInput")
    y0_d = nc.dram_tensor("y0", [128, S], F32, kind="ExternalInput")
    idz_d = nc.dram_tensor("idz", [128, 132], BF16, kind="ExternalInput")
    idf_d = nc.dram_tensor("idf", [128, 128], F32, kind="ExternalInput")
    id16_d = nc.dram_tensor("id16", [16, 16], F32, kind="ExternalInput")
    v_d = nc.dram_tensor("vout", [S, K, D], F32, kind="ExternalOutput")
    taps = {}
    if DEBUG:
        for nm, shape in [("d_t0", [128, K, S]), ("d_T2", [128, K, S]),
                          ("d_bt00", [128, SEC]), ("d_eT0", [128, CH * 128]),
                          ("d_cs0", [128, CH, SG, K]), ("d_y0", [128, 128]),
                          ("d_Y2", [128, G, SG, K]), ("d_s1", [128, KQ, D]),
                          ("d_vsb0", [128, KQ, D]), ("d_t1", [128, K, S]),
                          ("d_T21", [128, K, S]), ("d_bt20", [128, SEC]),
                          ("d_y20", [128, 128]), ("d_Y22", [128, G, SG, K]),
                          ("d_s2", [128, KQ, D])]:
            taps[nm] = nc.dram_tensor(nm, shape, F32, kind="ExternalOutput")

    with tile.TileContext(nc) as tc, ExitStack() as ctx:
        const = ctx.enter_context(tc.tile_pool(name="const", bufs=1))
        xap = ctx.enter_context(tc.tile_pool(name="xa", bufs=1))
        xbp = ctx.enter_context(tc.tile_pool(name="xb", bufs=1))
        etp = ctx.enter_context(tc.tile_pool(name="et", bufs=2))
        csp = ctx.enter_context(tc.tile_pool(name="cs", bufs=2))
        zp = ctx.enter_context(tc.tile_pool(name="z", bufs=3))
        y2p = ctx.enter_context(tc.tile_pool(name="y2", bufs=1))
        ysp = ctx.enter_context(tc.tile_pool(name="ys", bufs=2))
        sqp = ctx.enter_context(tc.tile_pool(name="sqp", bufs=1))
        t2p = ctx.enter_context(tc.tile_pool(name="t2", bufs=2))
        vt2p = ctx.enter_context(tc.tile_pool(name="vt2", bufs=1))
        # PSUM: every buf is one full 2KB bank; exactly 8 in total.
        ps_t = ctx.enter_context(tc.tile_pool(name="ps_t", bufs=1, space="PSUM"))
        ps_e = ctx.enter_context(tc.tile_pool(name="ps_e", bufs=2, space="PSUM"))
        ps_bt = ctx.enter_context(tc.tile_pool(name="ps_bt", bufs=3, space="PSUM"))
        ps_y = ctx.enter_context(tc.tile_pool(name="ps_y", bufs=1, space="PSUM"))
        ps_s = ctx.enter_context(tc.tile_pool(name="ps_s", bufs=1, space="PSUM"))

        # ---- constants, host-precomputed, loaded at top scheduler priority
        with tc.high_priority():
            wr = const.tile([128, K, D], F32)
            nc.sync.dma_start(out=wr, in_=wr_d[:])
            y0t = const.tile([128, S], F32)
            nc.sync.dma_start(out=y0t, in_=y0_d[:])
            wt = const.tile([32, K, 128], BF16)
            nc.sync.dma_start(out=wt, in_=wt_d[:])
            idz = const.tile([128, 132], BF16)
            nc.sync.dma_start(out=idz, in_=idz_d[:])
            id128f = const.tile([128, 128], F32)
            nc.sync.dma_start(out=id128f, in_=idf_d[:])
            id16 = const.tile([16, 16], F32)
            nc.sync.dma_start(out=id16, in_=id16_d[:])

        # ---- x loads, batched per sample-group across both HWDGE queues
        dma_engines = [nc.sync, nc.scalar]
        xb_g = []
        xa_g = []
        for g in range(G):
            tb = xbp.tile([128, SG, CH * 128], BF16, tag=f"xb{g}")
            dma_engines[g % 2].dma_start(out=tb, in_=xb_d[:, SG * g:SG * g + SG])
            xb_g.append(tb)
            ta = xap.tile([128, SG, CH, 128], BF16, tag=f"xa{g}")
            dma_engines[(g + 1) % 2].dma_start(out=ta,
                                               in_=xa_d[:, SG * g:SG * g + SG])
            xa_g.append(ta)

        # ---- iteration 0: s from host-precomputed uniform-softmax y0
        s_ps = ps_s.tile([128, KQ, D], F32, tag="s")
        # strips only fill 16 of each 32 partitions; init the rest for squash
        nc.vector.memset(s_ps, 0.0)
        for k in range(K):
            t, kq = k % 4, k // 4
            nc.tensor.matmul(s_ps[32 * t:32 * t + S, kq, :], y0t,
                             wr[:, k, :], start=True, stop=True,
                             tile_position=(0, 32 * t),
                             skip_group_check=True)
        t_ps = ps_t.tile([128, K, S], F32, tag="t")

        def t_pass(vsb, first, T2_prev=None):
            """vt transposes + t-matmul; T2 accumulates across passes in bf16.

            The strip layout is first collected to partition base 0 via
            SBUF->SBUF DMAs: PE transposes at non-zero row tile_positions
            hang the device (NRT_EXEC_UNIT_UNRECOVERABLE) unless used as a
            single ascending sequence, so all transposes run at (0, 0).
            """
            vsb0 = sqp.tile([16, 4, KQ, D], F32, tag="vsb0")
            with tc.high_priority():
                for t in range(4):
                    nc.sync.dma_start(out=vsb0[:, t],
                                      in_=vsb[32 * t:32 * t + S])
            vt_ps = ps_s.tile([32, K, S], F32, tag="s")
            for t in range(4):
                for kq in range(KQ):
                    k = 4 * kq + t
                    nc.tensor.transpose(out=vt_ps[:, k, :],
                                        in_=vsb0[:, t, kq, :],
                                        identity=id16)
            T2 = t2p.tile([128, K, S], BF16, tag="T2")
            if STAGE == 21:
                nc.vector.memset(T2, 0.0)
                return T2
            vt2 = vt2p.tile([32, K, S], BF16, tag="vt2")
            nc.vector.tensor_copy(out=vt2, in_=vt_ps)
            for k in range(K):
                nc.tensor.matmul(t_ps[:, k, :], wt[:, k, :], vt2[:, k, :],
                                 start=True, stop=True)
            if STAGE == 22:
                nc.vector.memset(T2, 0.0)
                return T2
            for h in (0, 1):  # split so early groups' b-matmuls unblock sooner
                hs = slice(8 * h, 8 * h + 8)
                if first:
                    nc.vector.tensor_copy(out=T2[:, :, hs], in_=t_ps[:, :, hs])
                else:
                    nc.vector.tensor_tensor(out=T2[:, :, hs], in0=t_ps[:, :, hs],
                                            in1=T2_prev[:, :, hs], op=ADD)
            if DEBUG and not first:
                d_t1sb = const.tile([128, K, S], F32, name="d_t1sb")
                nc.vector.tensor_copy(out=d_t1sb, in_=t_ps)
                nc.sync.dma_start(out=taps["d_t1"][:], in_=d_t1sb)
                d_T21sb = const.tile([128, K, S], F32, name="d_T21sb")
                nc.vector.tensor_copy(out=d_T21sb, in_=T2)
                nc.sync.dma_start(out=taps["d_T21"][:], in_=d_T21sb)
            return T2

        vsb = _squash(nc, sqp, s_ps, act_square=False)
        if DEBUG:
            nc.sync.dma_start(out=taps["d_vsb0"][:], in_=vsb)
        def emit_out(vsb):
            vq = v_d[:].rearrange("s (kq four) d -> four s kq d", four=4)
            with tc.high_priority():
                for t in range(4):
                    eng = nc.sync if t % 2 == 0 else nc.scalar
                    eng.dma_start(out=vq[t], in_=vsb[32 * t:32 * t + S])
        if STAGE == 1:
            emit_out(vsb)
        T2 = t_pass(vsb, first=True) if (STAGE >= 2 or STAGE in (21, 22)) else None
        if DEBUG:
            d_t0sb = const.tile([128, K, S], F32, name="d_t0sb")
            nc.vector.tensor_copy(out=d_t0sb, in_=t_ps)
            nc.sync.dma_start(out=taps["d_t0"][:], in_=d_t0sb)
            d_T2sb = const.tile([128, K, S], F32, name="d_T2sb")
            nc.vector.tensor_copy(out=d_T2sb, in_=T2)
            nc.sync.dma_start(out=taps["d_T2"][:], in_=d_T2sb)

        if STAGE in (2, 21, 22):
            emit_out(vsb)

        for p in ((1, 2) if STAGE >= 6 else ((1,) if (STAGE >= 3 and STAGE not in (21, 22)) else ())):
            eT_g = [None] * G
            bt_gs = [[None] * NSEC for _ in range(G)]

            def emit_b(g):
                eT = etp.tile([128, CH * 128], BF16, tag="eT")
                eT_g[g] = eT
                for sec in range(NSEC):
                    bt = ps_bt.tile([128, SEC], F32, tag="bt")
                    bt_gs[g][sec] = bt
                    for t in range(4):
                        s = SG * g + t
                        nc.tensor.matmul(bt[32 * t:32 * t + 32, :],
                                         T2[:, :, s],
                                         xb_g[g][:, t, sec * SEC:(sec + 1) * SEC],
                                         start=True, stop=True,
                                         tile_position=(0, 32 * t))
                    nc.scalar.activation(out=eT[:, sec * SEC:(sec + 1) * SEC],
                                         in_=bt, func=EXP)
                    if DEBUG and g == 0 and sec == 0:
                        d_btsb = const.tile([128, SEC], F32,
                                            name="d_btsb", tag=f"d_btsb{p}")
                        nc.vector.tensor_copy(out=d_btsb, in_=bt)
                        nm = "d_bt00" if p == 1 else "d_bt20"
                        nc.sync.dma_start(out=taps[nm][:], in_=d_btsb)

            y_ps = ps_y.tile([128, G, 128], F32, tag="y")

            def emit_smy(g):
                eT = eT_g[g]
                # transpose fused with Z: identity is [I128 | block-ones(4)],
                # so cols 128:132 of each transposed chunk hold the per-sample
                # k-sums (softmax denominators) computed by the PE for free.
                e_t = ps_e.tile([128, 3, 132], F32, tag="e")
                cs = csp.tile([128, CH, SG, K], BF16, tag="cs")
                for ic0 in range(0, CH, 3):
                    nsl = min(3, CH - ic0)
                    for ic in range(ic0, ic0 + nsl):
                        nc.tensor.matmul(
                            e_t[:, ic % 3],
                            eT[:, ic * 128:(ic + 1) * 128],
                            idz, start=True, stop=True)
                    sl0 = ic0 % 3
                    esl = e_t[:, sl0:sl0 + nsl, 0:128].rearrange(
                        "p n (sg k) -> p n sg k", sg=SG)
                    r = zp.tile([128, 3, SG], F32, tag="r")
                    nc.vector.reciprocal(out=r[:, 0:nsl],
                                         in_=e_t[:, sl0:sl0 + nsl, 128:132])
                    nc.vector.tensor_tensor(
                        out=cs[:, ic0:ic0 + nsl], in0=esl,
                        in1=r[:, 0:nsl].unsqueeze(-1).broadcast_to(
                            [128, nsl, SG, K]),
                        op=MULT)
                for ic in range(CH):
                    for t in range(4):
                        nc.tensor.matmul(y_ps[32 * t:32 * t + 32, g, :],
                                         cs[:, ic, t, :],
                                         xa_g[g][:, t, ic, :],
                                         start=(ic == 0), stop=(ic == CH - 1),
                                         tile_position=(0, 32 * t),
                                         skip_group_check=True)
                ysb = ysp.tile([128, 128], F32, tag="ysb")
                nc.vector.tensor_copy(out=ysb, in_=y_ps[:, g, :])
                nc.tensor.transpose(out=y_ps[:, g, :], in_=ysb,
                                    identity=id128f)
                nc.vector.tensor_copy(out=Y2[:, g].rearrange("p sg k -> p (sg k)"),
                                      in_=y_ps[:, g, :])
                if DEBUG and p == 2 and g == 0:
                    d_y2sb = const.tile([128, 128], F32, name="d_y2sb")
                    nc.vector.tensor_copy(out=d_y2sb, in_=y_ps[:, 0, :])
                    nc.sync.dma_start(out=taps["d_y20"][:], in_=d_y2sb)
                if DEBUG and p == 1 and g == 0:
                    d_eTsb = const.tile([128, CH * 128], F32, name="d_eTsb")
                    nc.vector.tensor_copy(out=d_eTsb, in_=eT)
                    nc.sync.dma_start(out=taps["d_eT0"][:], in_=d_eTsb)
                    d_cssb = const.tile([128, CH, SG, K], F32, name="d_cssb")
                    nc.vector.tensor_copy(out=d_cssb, in_=cs)
                    nc.sync.dma_start(out=taps["d_cs0"][:], in_=d_cssb)
                    d_ysb = const.tile([128, 128], F32, name="d_ysb")
                    nc.vector.tensor_copy(out=d_ysb, in_=y_ps[:, 0, :])
                    nc.sync.dma_start(out=taps["d_y0"][:], in_=d_ysb)

            Y2 = y2p.tile([128, G, SG, K], F32, tag="Y2")
            # software-pipelined emission: PE never waits on ACT exp
            if STAGE == 3:
                emit_b(0); emit_b(1); emit_b(2); emit_b(3)
                emit_out(vsb)
                break
            emit_b(0)
            emit_b(1)
            emit_smy(0)
            emit_b(2)
            emit_smy(1)
            emit_b(3)
            emit_smy(2)
            emit_smy(3)
            if STAGE == 4:
                emit_out(vsb)
                break

            # s-matmul, column-tiled: capsule k -> strip t=k%4, row kq=k//4
            if STAGE == 4:
                break
            s_ps = ps_s.tile([128, KQ, D], F32, tag="s")
            nc.vector.memset(s_ps, 0.0)
            for k in range(K):
                t, kq = k % 4, k // 4
                nc.tensor.matmul(s_ps[32 * t:32 * t + S, kq, :],
                                 Y2[:, :, :, k], wr[:, k, :],
                                 start=True, stop=True,
                                 tile_position=(0, 32 * t),
                                 skip_group_check=True)
            if DEBUG:
                nc.sync.dma_start(out=taps["d_Y2" if p == 1 else "d_Y22"][:],
                                  in_=Y2)
                d_s1sb = const.tile([128, KQ, D], F32,
                                    name="d_s1sb", tag=f"d_s1sb{p}")
                nc.vector.tensor_copy(out=d_s1sb, in_=s_ps)
                nc.sync.dma_start(out=taps["d_s1" if p == 1 else "d_s2"][:],
                                  in_=d_s1sb)
            vsb = _squash(nc, sqp, s_ps)
            if p == 1:
                T2 = t_pass(vsb, first=False, T2_prev=T2)
                if STAGE == 5:
                    emit_out(vsb)
            else:
                emit_out(vsb)

    nc.compile()
    return nc


def _get_program():
    global _PROGRAM
    if _PROGRAM is None:
        _PROGRAM = _build_program()
    return _PROGRAM


def _prep_core_inputs(x_core, wr, wt):
    """x_core: [S, I, J] fp32 -> per-core input map."""
    bf = ml_dtypes.bfloat16
    xa = np.ascontiguousarray(
        x_core.reshape(S, CH, 128, J).transpose(2, 0, 1, 3).astype(bf))  # [128,S,CH,J]
    xb = np.ascontiguousarray(x_core.transpose(2, 0, 1).astype(bf))      # [J,S,I]
    y0 = np.ascontiguousarray((x_core.sum(axis=1) / K).T)                # [J,S] f32
    return {"xa": xa, "xb": xb.reshape(J, S, CH * 128), "wr": wr,
            "wt": wt, "y0": y0,
            "idz": np.concatenate(
                [np.eye(128, dtype=np.float32),
                 np.kron(np.eye(4, dtype=np.float32), np.ones((32, 1), np.float32))],
                axis=1).astype(bf),
            "idf": np.eye(128, dtype=np.float32),
            "id16": np.eye(16, dtype=np.float32)}


def kernel(inputs, W):
    x = np.ascontiguousarray(np.asarray(inputs, dtype=np.float32))
    Wf = np.ascontiguousarray(np.asarray(W, dtype=np.float32))           # [J, K, D]
    wt = np.ascontiguousarray(
        Wf.transpose(2, 1, 0).astype(ml_dtypes.bfloat16))                # [D, K, J]
    nc = _get_program()
    in_maps = [_prep_core_inputs(x[c * S:(c + 1) * S], Wf, wt) for c in range(NCORES)]
    res = run_bass_kernel_spmd(nc, in_maps, list(range(NCORES)))
    return np.concatenate([r["vout"] for r in res.results], axis=0)
